# revision 31
# baseline (speedup 1.0000x reference)
"""Trainium2 Bass kernel for nn_Attention1D (B=4, L=4096, C=64).

reference:
    Q = x@Wq + bq ; K = x@Wk + bk ; V = x@Wv + bv          (per batch b)
    s = Q @ K.T / sqrt(C)                                   [L_q, L_k]
    attn = softmax(s, axis=q)      # normalize over QUERY axis
    out = attn @ V + x

Sharding: 8 cores = 4 batches x 2 key-shards. softmax normalizes over q
(not sharded) so each core's softmax is local: Z[k] = sum_q exp(s[q,k]),
out += exp(s) @ (V/Z); the two k-shards' partial outputs ADD on the host,
which also transposes the channel-major core output and adds residual x.
Core (b, 1) gets x^T rolled by -2048 so its k-shard is always chunks 0-3.

Phase-split design (sequential PSUM pools give each phase all 8 banks):
  Head: combined Wq|Wk DMA + x^T in 8 chunks + host-computed V (bf16,
    needed only at the end of phase 1); an early dummy ACTIVATE preloads
    the exp table; K/Q(0-3) projections evac through batched [128,1024]
    PSUM tiles -- Q copies on ScalarE, K casts on VectorE. Q(4-7) is
    projected later, staged through a conveyor ring slot, so the x^T DMA
    tail never gates the conveyor start.
  Phase 1 conveyor: per score chunk (k-tile, q-half): 4 unpacked score
    MMs (contract 65, N=512, f32r full-rate 427ns) fill a [128,2048]
    PSUM slot (2-slot ring); ScalarE drains each slot with one exp
    ACTIVATE (+Z-partial accum, bias -ln64 for fp8 headroom, ~2.2us).
    The conveyor rate is max(fill ~2.1, drain ~2.2) -- measured: engine-
    splitting the drains can NOT help (ring depth 2 = strict pipeline),
    and HAM-warm fills don't survive the ~0.3us fill gaps. ET lands as
    fp8e4 tiles [128, 2, 2048] per (k-tile-pair, q-half): the pair's
    chunks are contiguous halves = phase-2's 3D DoubleRow rhs AP.
  Phase 2: dense AV with V-as-weights in fp8 DoubleRow (2 k-tiles per
    MM, contract 256): outT[f,q] in 8 one-bank PSUM tiles [64,512],
    8 pair-MMs each, j-outer so each tile's evac+DMA overlap remaining
    MMs. A 10-MM dummy burst at the end of phase 1 keeps the PE busy
    through the pool boundary so the HAM clock-gate runs the whole AV
    warm (2.4GHz, 216ns per MM). Host divides by the gv 16x pre-scale
    (keeps fp8 gv out of the subnormal range) during the unshard.
"""

import numpy as np
import ml_dtypes  # noqa: F401  (np bf16 support registered on import)

B, L, C = 4, 4096, 64
NCORES = 8
KSH = L // 2          # k columns per core: 2048
NKT = KSH // 128      # 16 k-tiles per core
NQ5 = L // 512        # 8 q-chunks of 512
NCH = NKT * 2         # 32 score chunks of [128, 2048]

LN64 = float(np.log(64.0))  # exp pre-scale: e^s/64 keeps fp8e4 range (s<=10.2)

_cache = {}


def _build():
    import concourse.bacc as bacc
    import concourse.mybir as mybir
    import concourse.tile as tile

    bf16 = mybir.dt.bfloat16
    f8 = mybir.dt.float8e4
    f32 = mybir.dt.float32
    f32r = mybir.dt.float32r
    AF = mybir.ActivationFunctionType
    AX = mybir.AxisListType
    ALU = mybir.AluOpType

    nc = bacc.Bacc("TRN2", target_bir_lowering=False, debug=False)

    xt_d = nc.dram_tensor("xt", [C + 1, L], f32r, kind="ExternalInput")
    w_d = nc.dram_tensor("w", [C + 1, 2 * C], f32r, kind="ExternalInput")
    v_d = nc.dram_tensor("v", [128, NKT, C], mybir.dt.bfloat16, kind="ExternalInput")
    o_d = nc.dram_tensor("o", [C, L], f32, kind="ExternalOutput")

    with tile.TileContext(nc) as tc:
        with (
            tc.tile_pool(name="consts", bufs=1) as consts,
            tc.tile_pool(name="sb", bufs=1) as sb,
            tc.tile_pool(name="obp", bufs=2) as obp,
        ):
            # early exp-table preload: tiny ACTIVATE on a zeroed scratch
            scr = consts.tile([128, 8], f32)
            nc.vector.memset(scr, 0.0)
            nc.scalar.activation(out=scr, in_=scr, func=AF.Exp)

            wu = consts.tile([128, 512], bf16)   # warm-burst operand
            nc.vector.memset(wu, 0.0)

            bln16 = consts.tile([128, 1], f32)   # exp bias: -ln16
            nc.vector.memset(bln16, -LN64)

            w_s = consts.tile([C + 1, 2 * C], f32r)
            nc.sync.dma_start(out=w_s, in_=w_d.ap())
            wq_s = w_s[:, 0:C]
            wk_s = w_s[:, C:2 * C]

            xt_c = []
            for c in range(NQ5):
                t = sb.tile([C + 1, 512], f32r, tag=f"xt{c}")
                nc.sync.dma_start(out=t, in_=xt_d.ap()[:, c * 512:(c + 1) * 512])
                xt_c.append(t)

            qt_c = [sb.tile([64, 1024], f32r, tag=f"qt{c}", name=f"qt{c}")
                    for c in range(4)]
            kt_c = [sb.tile([64, 1024], f32r, tag=f"kt{c}", name=f"kt{c}")
                    for c in range(2)]
            v_all = sb.tile([128, NKT, C], bf16)
            nc.sync.dma_start(out=v_all, in_=v_d.ap())
            # ET in fp8e4, scaled by 1/16 (exp bias -ln16) for headroom;
            # one tile per (k-tile-PAIR, q-half): [128, 2, 2048] = the two
            # k-tiles' chunks as contiguous halves, so phase-2 DoubleRow
            # matmuls read them as a 3D [128, 2, 512] rhs AP.
            ep_ts = [sb.tile([128, 2, 2048], f8, tag=f"ep{c}", name=f"ep{c}")
                     for c in range(NKT)]
            zp = sb.tile([128, NCH], f32)     # Z partials per chunk
            z_all = sb.tile([128, NKT], f32)
            rz_all = sb.tile([128, NKT], f32)
            gv_all = sb.tile([128, NKT, C], f8)

            with tc.tile_pool(name="qkvp", bufs=1, space="PSUM") as qkvp:
                # K then Q interleaved as x^T chunks land; batched evacs:
                # Q copies on ScalarE, K/V casts on VectorE.
                for g in range(2):
                    pk = qkvp.tile([128, 1024], f32, tag="p", bufs=3,
                                   name=f"pk{g}")
                    for h in range(2):
                        nc.tensor.matmul(
                            pk[0:64, h * 512:(h + 1) * 512], lhsT=wk_s,
                            rhs=xt_c[2 * g + h], start=True, stop=True,
                        )
                    nc.vector.tensor_copy(out=kt_c[g], in_=pk[0:64, :])
                    pq = qkvp.tile([128, 1024], f32, tag="p", bufs=3,
                                   name=f"pq{g}")
                    for h in range(2):
                        nc.tensor.matmul(
                            pq[0:64, h * 512:(h + 1) * 512], lhsT=wq_s,
                            rhs=xt_c[2 * g + h], start=True, stop=True,
                        )
                    nc.scalar.copy(out=qt_c[g], in_=pq[0:64, :])
                # (V = x@Wv + bv comes from the host, DMA'd after x^T --
                # it is only needed for gv at the end of phase 1.)
                # (Q chunks 4-7 are projected later, staged through a
                # conveyor ring slot, so the DMA tail doesn't gate the
                # qkv pool close / conveyor start.)

            # ---------- phase 1: scores + exp conveyor ----------
            def z_batch(k0, k1):
                zv = zp[:, 2 * k0:2 * k1].rearrange("p (k h) -> p k h", h=2)
                nc.vector.reduce_sum(out=z_all[:, k0:k1], in_=zv, axis=AX.X)
                nc.vector.reciprocal(out=rz_all[:, k0:k1], in_=z_all[:, k0:k1])
                for k2 in range(k0, k1):
                    nc.vector.tensor_scalar(
                        out=gv_all[:, k2, :], in0=v_all[:, k2, :],
                        scalar1=rz_all[:, k2:k2 + 1], scalar2=16.0,
                        op0=ALU.mult, op1=ALU.mult,
                    )

            # Chunk order: q-half-0 chunks of k-tiles 0-3 first (they only
            # need x^T chunks 0-3), with the Q4-7 projection staged through
            # the ring after two fills (by then its x^T DMAs have landed).
            order = [(0, 0), (1, 0), "SQ", (2, 0), (3, 0),
                     (0, 1), (1, 1), (2, 1), (3, 1)]
            for kt in range(4, NKT):
                order += [(kt, 0), (kt, 1)]

            with tc.tile_pool(name="scp", bufs=2, space="PSUM") as scp:
                done = set()
                for item in order:
                    if item == "SQ":
                        Sq = scp.tile([128, 2048], f32, tag="s")
                        for cc in range(4, NQ5):
                            nc.tensor.matmul(
                                Sq[0:64, (cc - 4) * 512:(cc - 3) * 512],
                                lhsT=wq_s, rhs=xt_c[cc],
                                start=True, stop=True,
                            )
                        nc.vector.tensor_copy(out=qt_c[2], in_=Sq[0:64, 0:1024])
                        nc.vector.tensor_copy(out=qt_c[3],
                                              in_=Sq[0:64, 1024:2048])
                        continue
                    kt, qh = item
                    lk = kt_c[kt // 8][:, (kt % 8) * 128:(kt % 8 + 1) * 128]
                    S = scp.tile([128, 2048], f32, tag="s")
                    for c5 in range(4):
                        cc = qh * 4 + c5
                        nc.tensor.matmul(
                            S[:, c5 * 512:(c5 + 1) * 512],
                            lhsT=lk,
                            rhs=qt_c[cc // 2][:, (cc % 2) * 512:(cc % 2 + 1) * 512],
                            start=True, stop=True,
                        )
                    ci = kt * 2 + qh
                    nc.scalar.activation(
                        out=ep_ts[(kt // 2) * 2 + qh][:, kt % 2, :],
                        in_=S, func=AF.Exp, bias=bln16,
                        accum_out=zp[:, ci:ci + 1],
                    )
                    done.add(ci)
                    if all(c in done for c in range(26)) and 99 not in done:
                        done.add(99)
                        # early Z/rz/gv for k-tiles 0..12 (partials all in)
                        z_batch(0, 13)
                # keep the PE busy through the phase boundary (HAM warm)
                Sw = scp.tile([128, 2048], f32, tag="s")
                for i in range(10):
                    nc.tensor.matmul(
                        Sw[:, (i % 4) * 512:(i % 4 + 1) * 512],
                        lhsT=wu[:, 0:128], rhs=wu,
                        start=True, stop=True,
                    )

            z_batch(13, NKT)

            # ---------- phase 2: dense AV (V as weights, outT[f, q]) ----------
            o_ap = o_d.ap()
            with tc.tile_pool(name="accp", bufs=1, space="PSUM") as accp:
                acc = [accp.tile([128, 512], f32, tag=f"a{j}", name=f"a{j}")
                       for j in range(NQ5)]
                for j in range(NQ5):
                    qh, jj = j // 4, j % 4
                    for p in range(NKT // 2):
                        nc.tensor.matmul(
                            acc[j][0:64, :],
                            lhsT=gv_all[:, 2 * p:2 * p + 2, :],
                            rhs=ep_ts[2 * p + qh][:, :, jj * 512:(jj + 1) * 512],
                            start=(p == 0), stop=(p == NKT // 2 - 1),
                            perf_mode=mybir.MatmulPerfMode.DoubleRow,
                            skip_group_check=True,
                        )
                    ob = obp.tile([64, 512], f32, tag="ob")
                    nc.vector.tensor_copy(out=ob, in_=acc[j][0:64, :])
                    nc.sync.dma_start(
                        out=o_ap[:, j * 512:(j + 1) * 512], in_=ob,
                    )

    nc.compile()
    return nc


def _get_nc():
    if "nc" not in _cache:
        _cache["nc"] = _build()
    return _cache["nc"]


def _in_maps(x, Wq, bq, Wk, bk, Wv, bv):
    s = 1.0 / np.sqrt(np.float32(C))
    wq1 = (np.concatenate([Wq, bq[None, :]], 0) * s).astype(np.float32)
    wk1 = np.concatenate([Wk, bk[None, :]], 0).astype(np.float32)
    w = np.ascontiguousarray(np.concatenate([wq1, wk1], 1))
    maps = []
    for core in range(NCORES):
        b, half = core // 2, core % 2
        x1t = np.ascontiguousarray(np.concatenate(
            [x[b], np.ones((L, 1), np.float32)], 1
        ).T.astype(np.float32))              # [65, L]
        if half == 1:
            x1t = np.ascontiguousarray(np.roll(x1t, -KSH, axis=1))
        vsh = (x[b] @ Wv + bv)[half * KSH:(half + 1) * KSH]   # [KSH, C]
        v = np.ascontiguousarray(
            vsh.reshape(NKT, 128, C).transpose(1, 0, 2)
        ).astype(ml_dtypes.bfloat16)                          # [128, NKT, C]
        maps.append({"xt": x1t, "w": w, "v": v})
    return maps


def _unshard(outs, x):
    full = np.empty((B, L, C), np.float32)
    for b in range(B):
        o0 = outs[2 * b].astype(np.float32)       # [C, L]
        o1 = outs[2 * b + 1].astype(np.float32)   # [C, L] rolled by -KSH
        o1 = np.roll(o1, KSH, axis=1)
        full[b] = (o0 + o1).T * (1.0 / 16.0) + x[b]
    return full


def _run(x, Wq, bq, Wk, bk, Wv, bv, trace=False):
    from concourse.bass_utils import run_bass_kernel_spmd

    nc = _get_nc()
    maps = _in_maps(x, Wq, bq, Wk, bk, Wv, bv)
    res = run_bass_kernel_spmd(
        nc, maps, core_ids=list(range(NCORES)), trace=trace
    )
    outs = [r["o"] for r in res.results]
    return _unshard(outs, x), res


def kernel(x, Wq, bq, Wk, bk, Wv, bv):
    x = np.asarray(x, np.float32)
    full, _ = _run(
        x,
        np.asarray(Wq, np.float32), np.asarray(bq, np.float32),
        np.asarray(Wk, np.float32), np.asarray(bk, np.float32),
        np.asarray(Wv, np.float32), np.asarray(bv, np.float32),
    )
    return full


# revision 32
# speedup vs baseline: 1.0063x; 1.0063x over previous
"""Trainium2 Bass kernel for nn_Attention1D (B=4, L=4096, C=64).

reference:
    Q = x@Wq + bq ; K = x@Wk + bk ; V = x@Wv + bv          (per batch b)
    s = Q @ K.T / sqrt(C)                                   [L_q, L_k]
    attn = softmax(s, axis=q)      # normalize over QUERY axis
    out = attn @ V + x

Sharding: 8 cores = 4 batches x 2 key-shards. softmax normalizes over q
(not sharded) so each core's softmax is local: Z[k] = sum_q exp(s[q,k]),
out += exp(s) @ (V/Z); the two k-shards' partial outputs ADD on the host,
which also transposes the channel-major core output and adds residual x.
Core (b, 1) gets x^T rolled by -2048 so its k-shard is always chunks 0-3.

Phase-split design (sequential PSUM pools give each phase all 8 banks):
  Head: combined Wq|Wk DMA + x^T in 8 chunks + host-computed V (bf16,
    needed only at the end of phase 1); an early dummy ACTIVATE preloads
    the exp table; K/Q(0-3) projections evac through batched [128,1024]
    PSUM tiles -- Q copies on ScalarE, K casts on VectorE. Q(4-7) is
    projected later, staged through a conveyor ring slot, so the x^T DMA
    tail never gates the conveyor start.
  Phase 1 conveyor: per score chunk (k-tile, q-half): 4 unpacked score
    MMs (contract 65, N=512, f32r full-rate 427ns) fill a [128,2048]
    PSUM slot (2-slot ring); ScalarE drains each slot with one exp
    ACTIVATE (+Z-partial accum, bias -ln64 for fp8 headroom, ~2.2us).
    The conveyor rate is max(fill ~2.1, drain ~2.2) -- measured: engine-
    splitting the drains can NOT help (ring depth 2 = strict pipeline),
    and HAM-warm fills don't survive the ~0.3us fill gaps. ET lands as
    fp8e4 tiles [128, 2, 2048] per (k-tile-pair, q-half): the pair's
    chunks are contiguous halves = phase-2's 3D DoubleRow rhs AP.
  Phase 2: dense AV with V-as-weights in fp8 DoubleRow (2 k-tiles per
    MM, contract 256): outT[f,q] in 8 one-bank PSUM tiles [64,512],
    8 pair-MMs each, j-outer so each tile's evac+DMA overlap remaining
    MMs. A 10-MM dummy burst at the end of phase 1 keeps the PE busy
    through the pool boundary so the HAM clock-gate runs the whole AV
    warm (2.4GHz, 216ns per MM). Host divides by the gv 16x pre-scale
    (keeps fp8 gv out of the subnormal range) during the unshard.
"""

import numpy as np
import ml_dtypes  # noqa: F401  (np bf16 support registered on import)

B, L, C = 4, 4096, 64
NCORES = 8
KSH = L // 2          # k columns per core: 2048
NKT = KSH // 128      # 16 k-tiles per core
NQ5 = L // 512        # 8 q-chunks of 512
NCH = NKT * 2         # 32 score chunks of [128, 2048]

LN64 = float(np.log(64.0))  # exp pre-scale: e^s/64 keeps fp8e4 range (s<=10.2)

_cache = {}


def _build():
    import concourse.bacc as bacc
    import concourse.mybir as mybir
    import concourse.tile as tile

    bf16 = mybir.dt.bfloat16
    f8 = mybir.dt.float8e4
    f32 = mybir.dt.float32
    f32r = mybir.dt.float32r
    AF = mybir.ActivationFunctionType
    AX = mybir.AxisListType
    ALU = mybir.AluOpType

    nc = bacc.Bacc("TRN2", target_bir_lowering=False, debug=False)

    xt_d = nc.dram_tensor("xt", [C + 1, L], f32r, kind="ExternalInput")
    w_d = nc.dram_tensor("w", [C + 1, 2 * C], f32r, kind="ExternalInput")
    v_d = nc.dram_tensor("v", [128, NKT, C], mybir.dt.bfloat16, kind="ExternalInput")
    o_d = nc.dram_tensor("o", [C, L], f32, kind="ExternalOutput")

    with tile.TileContext(nc) as tc:
        with (
            tc.tile_pool(name="consts", bufs=1) as consts,
            tc.tile_pool(name="sb", bufs=1) as sb,
            tc.tile_pool(name="obp", bufs=2) as obp,
        ):
            # early exp-table preload: tiny ACTIVATE on a zeroed scratch
            scr = consts.tile([128, 8], f32)
            nc.vector.memset(scr, 0.0)
            nc.scalar.activation(out=scr, in_=scr, func=AF.Exp)

            wu = consts.tile([128, 512], bf16)   # warm-burst operand
            nc.vector.memset(wu, 0.0)

            bln16 = consts.tile([128, 1], f32)   # exp bias: -ln16
            nc.vector.memset(bln16, -LN64)

            w_s = consts.tile([C + 1, 2 * C], f32r)
            nc.sync.dma_start(out=w_s, in_=w_d.ap())
            wq_s = w_s[:, 0:C]
            wk_s = w_s[:, C:2 * C]

            xt_c = []
            for c in range(NQ5):
                t = sb.tile([C + 1, 512], f32r, tag=f"xt{c}")
                nc.sync.dma_start(out=t, in_=xt_d.ap()[:, c * 512:(c + 1) * 512])
                xt_c.append(t)

            qt_c = [sb.tile([64, 1024], f32r, tag=f"qt{c}", name=f"qt{c}")
                    for c in range(4)]
            kt_c = [sb.tile([64, 1024], f32r, tag=f"kt{c}", name=f"kt{c}")
                    for c in range(2)]
            v_all = sb.tile([128, NKT, C], bf16)
            nc.sync.dma_start(out=v_all, in_=v_d.ap())
            # ET in fp8e4, scaled by 1/16 (exp bias -ln16) for headroom;
            # one tile per (k-tile-PAIR, q-half): [128, 2, 2048] = the two
            # k-tiles' chunks as contiguous halves, so phase-2 DoubleRow
            # matmuls read them as a 3D [128, 2, 512] rhs AP.
            ep_ts = [sb.tile([128, 2, 2048], f8, tag=f"ep{c}", name=f"ep{c}")
                     for c in range(NKT)]
            zp = sb.tile([128, NCH], f32)     # Z partials per chunk
            z_all = sb.tile([128, NKT], f32)
            rz_all = sb.tile([128, NKT], f32)
            gv_all = sb.tile([128, NKT, C], f8)

            with tc.tile_pool(name="qkvp", bufs=1, space="PSUM") as qkvp:
                # K then Q interleaved as x^T chunks land; batched evacs:
                # Q copies on ScalarE, K/V casts on VectorE.
                for g in range(2):
                    pk = qkvp.tile([128, 1024], f32, tag="p", bufs=4,
                                   name=f"pk{g}")
                    for h in range(2):
                        nc.tensor.matmul(
                            pk[0:64, h * 512:(h + 1) * 512], lhsT=wk_s,
                            rhs=xt_c[2 * g + h], start=True, stop=True,
                        )
                    nc.vector.tensor_copy(out=kt_c[g], in_=pk[0:64, :])
                    pq = qkvp.tile([128, 1024], f32, tag="p", bufs=4,
                                   name=f"pq{g}")
                    for h in range(2):
                        nc.tensor.matmul(
                            pq[0:64, h * 512:(h + 1) * 512], lhsT=wq_s,
                            rhs=xt_c[2 * g + h], start=True, stop=True,
                        )
                    nc.scalar.copy(out=qt_c[g], in_=pq[0:64, :])
                # (V = x@Wv + bv comes from the host, DMA'd after x^T --
                # it is only needed for gv at the end of phase 1.)
                # (Q chunks 4-7 are projected later, staged through a
                # conveyor ring slot, so the DMA tail doesn't gate the
                # qkv pool close / conveyor start.)

            # ---------- phase 1: scores + exp conveyor ----------
            def z_batch(k0, k1):
                zv = zp[:, 2 * k0:2 * k1].rearrange("p (k h) -> p k h", h=2)
                nc.vector.reduce_sum(out=z_all[:, k0:k1], in_=zv, axis=AX.X)
                nc.vector.reciprocal(out=rz_all[:, k0:k1], in_=z_all[:, k0:k1])
                for k2 in range(k0, k1):
                    nc.vector.tensor_scalar(
                        out=gv_all[:, k2, :], in0=v_all[:, k2, :],
                        scalar1=rz_all[:, k2:k2 + 1], scalar2=16.0,
                        op0=ALU.mult, op1=ALU.mult,
                    )

            # Chunk order: q-half-0 chunks of k-tiles 0-3 first (they only
            # need x^T chunks 0-3), with the Q4-7 projection staged through
            # the ring after two fills (by then its x^T DMAs have landed).
            order = [(0, 0), (1, 0), "SQ", (2, 0), (3, 0),
                     (0, 1), (1, 1), (2, 1), (3, 1)]
            for kt in range(4, NKT):
                order += [(kt, 0), (kt, 1)]

            with tc.tile_pool(name="scp", bufs=2, space="PSUM") as scp:
                done = set()
                for item in order:
                    if item == "SQ":
                        Sq = scp.tile([128, 2048], f32, tag="s")
                        for cc in range(4, NQ5):
                            nc.tensor.matmul(
                                Sq[0:64, (cc - 4) * 512:(cc - 3) * 512],
                                lhsT=wq_s, rhs=xt_c[cc],
                                start=True, stop=True,
                            )
                        nc.vector.tensor_copy(out=qt_c[2], in_=Sq[0:64, 0:1024])
                        nc.vector.tensor_copy(out=qt_c[3],
                                              in_=Sq[0:64, 1024:2048])
                        continue
                    kt, qh = item
                    lk = kt_c[kt // 8][:, (kt % 8) * 128:(kt % 8 + 1) * 128]
                    S = scp.tile([128, 2048], f32, tag="s")
                    for c5 in range(4):
                        cc = qh * 4 + c5
                        nc.tensor.matmul(
                            S[:, c5 * 512:(c5 + 1) * 512],
                            lhsT=lk,
                            rhs=qt_c[cc // 2][:, (cc % 2) * 512:(cc % 2 + 1) * 512],
                            start=True, stop=True,
                        )
                    ci = kt * 2 + qh
                    nc.scalar.activation(
                        out=ep_ts[(kt // 2) * 2 + qh][:, kt % 2, :],
                        in_=S, func=AF.Exp, bias=bln16,
                        accum_out=zp[:, ci:ci + 1],
                    )
                    done.add(ci)
                    if all(c in done for c in range(26)) and 99 not in done:
                        done.add(99)
                        # early Z/rz/gv for k-tiles 0..12 (partials all in)
                        z_batch(0, 13)
                # keep the PE busy through the phase boundary (HAM warm)
                Sw = scp.tile([128, 2048], f32, tag="s")
                for i in range(10):
                    nc.tensor.matmul(
                        Sw[:, (i % 4) * 512:(i % 4 + 1) * 512],
                        lhsT=wu[:, 0:128], rhs=wu,
                        start=True, stop=True,
                    )

            z_batch(13, NKT)

            # ---------- phase 2: dense AV (V as weights, outT[f, q]) ----------
            o_ap = o_d.ap()
            with tc.tile_pool(name="accp", bufs=1, space="PSUM") as accp:
                acc = [accp.tile([128, 512], f32, tag=f"a{j}", name=f"a{j}")
                       for j in range(NQ5)]
                for j in range(NQ5):
                    qh, jj = j // 4, j % 4
                    for p in range(NKT // 2):
                        nc.tensor.matmul(
                            acc[j][0:64, :],
                            lhsT=gv_all[:, 2 * p:2 * p + 2, :],
                            rhs=ep_ts[2 * p + qh][:, :, jj * 512:(jj + 1) * 512],
                            start=(p == 0), stop=(p == NKT // 2 - 1),
                            perf_mode=mybir.MatmulPerfMode.DoubleRow,
                            skip_group_check=True,
                        )
                    ob = obp.tile([64, 512], f32, tag="ob")
                    nc.vector.tensor_copy(out=ob, in_=acc[j][0:64, :])
                    nc.sync.dma_start(
                        out=o_ap[:, j * 512:(j + 1) * 512], in_=ob,
                    )

    nc.compile()
    return nc


def _get_nc():
    if "nc" not in _cache:
        _cache["nc"] = _build()
    return _cache["nc"]


def _in_maps(x, Wq, bq, Wk, bk, Wv, bv):
    s = 1.0 / np.sqrt(np.float32(C))
    wq1 = (np.concatenate([Wq, bq[None, :]], 0) * s).astype(np.float32)
    wk1 = np.concatenate([Wk, bk[None, :]], 0).astype(np.float32)
    w = np.ascontiguousarray(np.concatenate([wq1, wk1], 1))
    maps = []
    for core in range(NCORES):
        b, half = core // 2, core % 2
        x1t = np.ascontiguousarray(np.concatenate(
            [x[b], np.ones((L, 1), np.float32)], 1
        ).T.astype(np.float32))              # [65, L]
        if half == 1:
            x1t = np.ascontiguousarray(np.roll(x1t, -KSH, axis=1))
        vsh = (x[b] @ Wv + bv)[half * KSH:(half + 1) * KSH]   # [KSH, C]
        v = np.ascontiguousarray(
            vsh.reshape(NKT, 128, C).transpose(1, 0, 2)
        ).astype(ml_dtypes.bfloat16)                          # [128, NKT, C]
        maps.append({"xt": x1t, "w": w, "v": v})
    return maps


def _unshard(outs, x):
    full = np.empty((B, L, C), np.float32)
    for b in range(B):
        o0 = outs[2 * b].astype(np.float32)       # [C, L]
        o1 = outs[2 * b + 1].astype(np.float32)   # [C, L] rolled by -KSH
        o1 = np.roll(o1, KSH, axis=1)
        full[b] = (o0 + o1).T * (1.0 / 16.0) + x[b]
    return full


def _run(x, Wq, bq, Wk, bk, Wv, bv, trace=False):
    from concourse.bass_utils import run_bass_kernel_spmd

    nc = _get_nc()
    maps = _in_maps(x, Wq, bq, Wk, bk, Wv, bv)
    res = run_bass_kernel_spmd(
        nc, maps, core_ids=list(range(NCORES)), trace=trace
    )
    outs = [r["o"] for r in res.results]
    return _unshard(outs, x), res


def kernel(x, Wq, bq, Wk, bk, Wv, bv):
    x = np.asarray(x, np.float32)
    full, _ = _run(
        x,
        np.asarray(Wq, np.float32), np.asarray(bq, np.float32),
        np.asarray(Wk, np.float32), np.asarray(bk, np.float32),
        np.asarray(Wv, np.float32), np.asarray(bv, np.float32),
    )
    return full
